# revision 1
# baseline (speedup 1.0000x reference)
"""Causal self-attention (B=2, T=2048, C=2048, H=16, D=128) on 8 trn2 cores.

Sharding: tensor-parallel over heads x data-parallel over batch.
Core c handles batch c//4, heads [4*(c%4) .. 4*(c%4)+4). Each core computes
qkv projection for its 4 heads, RoPE, causal attention, and a partial
output projection (its heads' rows of W_proj); the host sums the 4 partials
per batch.

Kernel structure (per core), all matmuls in fp32r (full PE rate @ N=512):
  Phase 1: QKV projection.
    Q,K produced transposed  (Q^T[d,t] = W_q^T x^T)  -> RoPE fused into the
    PSUM evacuation on DVE -> spilled to DRAM scratch.
    V produced natural      (V[t,d]   = x W_v)       -> DRAM scratch.
    All big DMAs split per k-tile so matmuls start as chunks land.
  Phase 2: attention per head, S^T orientation:
    S^T[k,q] = K^T.T @ Q^T   (one 128x512 matmul per tile, no contraction loop)
    causal mask on diagonal tiles = additive -1e30 on PSUM pre-exp (DVE)
    P^T = exp(S^T * 1/sqrt(D))  on ACT (no max subtraction -- scores are O(5))
    denominators: ones^T @ P^T accumulated in a [1,512] PSUM bank (PE)
    O^T[d,q] += V.T @ P^T    accumulated in PSUM over k-blocks
    normalize on evacuation: O^T * recip(broadcast(denom))
  Phase 3: partial proj: out[t,c] = sum_h O_h^T.T @ Wp_h   (O^T is already
    the required lhsT layout -- the whole kernel needs zero transposes).
"""

import contextlib
import math
import os

import numpy as np

B, T, C = 2, 2048, 2048
H, D = 16, 128
HPC = 4  # heads per core
NCORES = 8

_CACHE = {}


def _build_program():
    import concourse.tile as tile
    from concourse import bacc, mybir

    f32 = mybir.dt.float32
    f32r = mybir.dt.float32r
    Exp = mybir.ActivationFunctionType.Exp
    SCALE = 1.0 / math.sqrt(float(D))

    nc = bacc.Bacc(
        "TRN2", target_bir_lowering=False, debug=False, num_devices=NCORES
    )

    xT = nc.dram_tensor("xT", [C, T], f32r, kind="ExternalInput").ap()
    wqk = nc.dram_tensor("wqk", [C, 8 * 128], f32r, kind="ExternalInput").ap()
    wv = nc.dram_tensor("wv", [C, HPC * D], f32r, kind="ExternalInput").ap()
    wp = nc.dram_tensor("wp", [HPC * D, C], f32r, kind="ExternalInput").ap()
    onesr = nc.dram_tensor("onesr", [128, 128], f32r, kind="ExternalInput").ap()
    cosT = nc.dram_tensor("cosT", [128, T], f32, kind="ExternalInput").ap()
    sinTs = nc.dram_tensor("sinTs", [128, T], f32, kind="ExternalInput").ap()
    masks = nc.dram_tensor("masks", [4, 128, 512], f32, kind="ExternalInput").ap()
    out = nc.dram_tensor("out", [T, C], f32, kind="ExternalOutput").ap()

    KT = C // 128  # 16 contraction tiles
    NTB = T // 512  # 4 t-blocks

    with tile.TileContext(nc) as tc:
        with (
            tc.tile_pool(name="consts", bufs=1) as consts,
            tc.tile_pool(name="dram", bufs=1, space="DRAM") as dramp,
        ):
            es = contextlib.ExitStack()
            p2stp = es.enter_context(
                tc.tile_pool(name="p2st", bufs=5, space="PSUM")
            )
            cos_sb = consts.tile([128, T], f32, tag="cos")
            nc.sync.dma_start(out=cos_sb, in_=cosT)
            sin_sb = consts.tile([128, T], f32, tag="sin")
            nc.sync.dma_start(out=sin_sb, in_=sinTs)
            ones_sb = consts.tile([128, 128], f32r, tag="ones")
            nc.sync.dma_start(out=ones_sb, in_=onesr)

            # Per-chunk DRAM scratch so phase-2 loads can chase phase-1
            # writes chunk-by-chunk instead of waiting for whole tensors.
            qkt_dram = [
                [
                    dramp.tile([128, 512], f32r, tag=f"qkt{m}_{tb}",
                               name=f"qkt{m}_{tb}")
                    for tb in range(NTB)
                ]
                for m in range(8)
            ]
            vsc_dram = [
                dramp.tile([128, HPC * D], f32r, tag=f"vsc{i}", name=f"vsc{i}")
                for i in range(T // 128)
            ]

            # ---------------- Phase 1: QKV projection ----------------
            with (
                tc.tile_pool(name="p1x", bufs=2) as p1x,
                tc.tile_pool(name="p1w", bufs=1) as p1w,
                tc.tile_pool(name="p1wv", bufs=1) as p1wv,
                tc.tile_pool(name="p1e", bufs=2) as p1e,
                tc.tile_pool(name="p1ps", bufs=2, space="PSUM") as p1ps,
            ):
                # All 8 q/k weight M-tiles resident (64KB/part); chunked per k
                # and interleaved with the first x block so the first matmul
                # chain starts after ~2 chunks instead of the whole preload.
                wqkg = p1w.tile([128, KT, 8, 128], f32r, tag="wqkg")
                wv_sb = p1wv.tile([128, KT, HPC * D], f32r, tag="wv")
                xtb0 = p1x.tile([128, KT, 512], f32r, tag="xtb")
                MORD = (0, 4, 1, 5, 2, 6, 3, 7)

                def load_wm(m):
                    nc.sync.dma_start(
                        out=wqkg[:, :, m, :],
                        in_=wqk[:, m * 128 : (m + 1) * 128].rearrange(
                            "(k p) c -> p k c", p=128
                        ),
                    )

                # First compute chain (m=0) needs just its own weight column
                # and the first x chunks; stream the rest behind it.
                load_wm(MORD[0])
                load_wm(MORD[1])
                for k in range(KT):
                    nc.sync.dma_start(
                        out=xtb0[:, k], in_=xT[k * 128 : (k + 1) * 128, 0:512]
                    )
                    if k % 2 == 0 and k // 2 + 2 < 8:
                        load_wm(MORD[k // 2 + 2])
                for k in range(KT):
                    nc.sync.dma_start(
                        out=wv_sb[:, k], in_=wv[k * 128 : (k + 1) * 128, :]
                    )
                for tb in range(NTB):
                    tsl = slice(tb * 512, (tb + 1) * 512)
                    if tb == 0:
                        xtb = xtb0
                    else:
                        xtb = p1x.tile([128, KT, 512], f32r, tag="xtb",
                                       name="xtb")
                        for k in range(KT):
                            nc.sync.dma_start(
                                out=xtb[:, k],
                                in_=xT[k * 128 : (k + 1) * 128, tsl],
                            )
                    for m in (0, 4, 1, 5, 2, 6, 3, 7):
                        ps = p1ps.tile([128, 512], f32, tag="qk")
                        for k in range(KT):
                            nc.tensor.matmul(
                                ps,
                                lhsT=wqkg[:, k, m, :],
                                rhs=xtb[:, k, :],
                                start=(k == 0),
                                stop=(k == KT - 1),
                            )
                        # RoPE fused with PSUM evacuation.
                        qk_sb = p1e.tile([128, 512], f32r, tag="qke")
                        tmp = p1e.tile([128, 512], f32, tag="rtmp")
                        nc.vector.tensor_mul(
                            tmp[0:64], ps[64:128], sin_sb[0:64, tsl]
                        )
                        nc.vector.tensor_mul(
                            tmp[64:128], ps[0:64], sin_sb[64:128, tsl]
                        )
                        nc.vector.tensor_mul(qk_sb, ps, cos_sb[:, tsl])
                        nc.vector.tensor_add(qk_sb, qk_sb, tmp)
                        nc.scalar.dma_start(out=qkt_dram[m][tb], in_=qk_sb)
                    for tsub in range(4):
                        csl = slice(tsub * 128, (tsub + 1) * 128)
                        psv = p1ps.tile([128, 512], f32, tag="v", bufs=1)
                        for k in range(KT):
                            nc.tensor.matmul(
                                psv,
                                lhsT=xtb[:, k, csl],
                                rhs=wv_sb[:, k, :],
                                start=(k == 0),
                                stop=(k == KT - 1),
                            )
                        v_sb = p1e.tile([128, 512], f32r, tag="ve")
                        nc.scalar.copy(v_sb, psv)
                        nc.scalar.dma_start(
                            out=vsc_dram[tb * 4 + tsub], in_=v_sb
                        )

            # ---------------- Phases 2+3 share the O^T tiles -------------
            with tc.tile_pool(name="o2", bufs=1) as o2p:
                out2T = [
                    o2p.tile([128, T], f32r, tag=f"o2_{h}", name=f"o2_{h}")
                    for h in range(HPC)
                ]
                _phase2(tc, nc, f32, f32r, Exp, SCALE, KT, qkt_dram,
                        vsc_dram, masks, ones_sb, out2T, p2stp)
                es.close()
                _phase3(tc, nc, f32, f32r, out2T, wp, out)
    nc.compile()
    return nc


def _phase2(tc, nc, f32, f32r, Exp, SCALE, KT, qkt_dram, vsc_dram, masks,
            ones_sb, out2T, p2stp):
    with (
        tc.tile_pool(name="p2m", bufs=1) as p2m,
        tc.tile_pool(name="p2qkv", bufs=2) as p2qkv,
        tc.tile_pool(name="p2pt", bufs=3) as p2pt,
        tc.tile_pool(name="p2s", bufs=2) as p2s,
        tc.tile_pool(name="p2pv", bufs=2, space="PSUM") as p2pv,
        tc.tile_pool(name="p2dn", bufs=1, space="PSUM") as p2dn,
    ):
        mask_sb = p2m.tile([128, 4, 512], f32, tag="mask")
        nc.sync.dma_start(out=mask_sb, in_=masks.rearrange("j p q -> p j q"))
        for h in range(HPC):
            qt = p2qkv.tile([128, T], f32r, tag="qt")
            kt = p2qkv.tile([128, T], f32r, tag="kt")
            vt = p2qkv.tile([128, KT, 128], f32r, tag="vt")
            for tb in range(4):
                s = slice(tb * 512, (tb + 1) * 512)
                nc.sync.dma_start(out=kt[:, s], in_=qkt_dram[4 + h][tb])
                nc.sync.dma_start(out=qt[:, s], in_=qkt_dram[h][tb])
                for tsub in range(4):
                    i = tb * 4 + tsub
                    nc.sync.dma_start(
                        out=vt[:, i],
                        in_=vsc_dram[i][:, h * 128 : (h + 1) * 128],
                    )
            for qb in range(4):  # ascending: chases phase-1 output chunks
                qsl = slice(qb * 512, (qb + 1) * 512)
                pv = p2pv.tile([128, 512], f32, tag="pv")
                dn = p2dn.tile([128, 512], f32, tag="dn")
                nk = 4 * (qb + 1)
                for kb in range(nk):
                    st = p2stp.tile([128, 512], f32, tag="st")
                    nc.tensor.matmul(
                        st,
                        lhsT=kt[:, kb * 128 : (kb + 1) * 128],
                        rhs=qt[:, qsl],
                        start=True,
                        stop=True,
                    )
                    if kb >= qb * 4:
                        nc.vector.tensor_add(st, st, mask_sb[:, kb - qb * 4, :])
                    pt = p2pt.tile([128, 512], f32r, tag="pt")
                    nc.scalar.activation(pt, st, Exp, scale=SCALE)
                    nc.tensor.matmul(
                        dn,
                        lhsT=ones_sb,
                        rhs=pt,
                        start=(kb == 0),
                        stop=(kb == nk - 1),
                    )
                    nc.tensor.matmul(
                        pv,
                        lhsT=vt[:, kb, :],
                        rhs=pt,
                        start=(kb == 0),
                        stop=(kb == nk - 1),
                    )
                # dn already holds the denominator on every partition
                # (ones[128,128] lhsT): reciprocal + normalize, no broadcast.
                rb2 = p2s.tile([128, 512], f32, tag="rb2")
                nc.vector.reciprocal_approx_fast(out=rb2, in_=dn)
                nc.vector.tensor_mul(out2T[h][:, qsl], pv, rb2)


def _phase3(tc, nc, f32, f32r, out2T, wp, out):
    with (
        tc.tile_pool(name="p3w", bufs=1) as p3w,
        tc.tile_pool(name="p3e", bufs=4) as p3e,
        tc.tile_pool(name="p3ps", bufs=8, space="PSUM") as p3ps,
    ):
        wps = [
            p3w.tile([128, T], f32r, tag=f"wp{i}", name=f"wp{i}")
            for i in range(HPC)
        ]
        for i in range(HPC):
            nc.sync.dma_start(out=wps[i], in_=wp[i * 128 : (i + 1) * 128, :])
        for t in range(T // 128):
            tsl = slice(t * 128, (t + 1) * 128)
            pos = [
                p3ps.tile([128, 512], f32, tag="po", name=f"po{t}_{cb}")
                for cb in range(4)
            ]
            # hd outer / cb inner: 4 matmuls share one LDWEIGHTS.
            for hd in range(HPC):
                for cb in range(4):
                    nc.tensor.matmul(
                        pos[cb],
                        lhsT=out2T[hd][:, tsl],
                        rhs=wps[hd][:, cb * 512 : (cb + 1) * 512],
                        start=(hd == 0),
                        stop=(hd == HPC - 1),
                    )
            for cb in range(4):
                ob = p3e.tile([128, 512], f32, tag="ob")
                nc.vector.tensor_copy(ob, pos[cb])
                nc.sync.dma_start(
                    out=out[tsl, cb * 512 : (cb + 1) * 512], in_=ob
                )


def _get_program():
    if "nc" not in _CACHE:
        _CACHE["nc"] = _build_program()
    return _CACHE["nc"]


def make_in_maps(x, cos, sin, W_qkv, W_proj):
    """Host-side sharding: per-core input dicts (numpy, fp32)."""
    x = np.asarray(x, dtype=np.float32)
    cos = np.asarray(cos, dtype=np.float32)
    sin = np.asarray(sin, dtype=np.float32)
    W_qkv = np.asarray(W_qkv, dtype=np.float32)
    W_proj = np.asarray(W_proj, dtype=np.float32)

    cosT = np.ascontiguousarray(np.tile(cos.T, (2, 1)))  # [128, T]
    sinT = np.ascontiguousarray(np.concatenate([-sin.T, sin.T], axis=0))
    q_idx = np.arange(512)[None, None, :]
    k_idx = np.arange(128)[None, :, None]
    j_idx = np.arange(4)[:, None, None]
    masks = np.where(
        q_idx >= j_idx * 128 + k_idx, 0.0, -1.0e30
    ).astype(np.float32)  # [4, 128, 512] additive
    onesr = np.ones((128, 128), dtype=np.float32)

    in_maps = []
    for core in range(NCORES):
        b, hg = core // 4, core % 4
        csl = slice(hg * 512, (hg + 1) * 512)
        wqk_np = np.ascontiguousarray(
            np.concatenate(
                [W_qkv[:, csl], W_qkv[:, C + hg * 512 : C + (hg + 1) * 512]],
                axis=1,
            )
        )
        wv_np = np.ascontiguousarray(
            W_qkv[:, 2 * C + hg * 512 : 2 * C + (hg + 1) * 512]
        )
        wp_np = np.ascontiguousarray(W_proj[hg * 512 : (hg + 1) * 512, :])
        xT_np = np.ascontiguousarray(x[b].T)
        in_maps.append(
            {
                "xT": xT_np,
                "wqk": wqk_np,
                "wv": wv_np,
                "wp": wp_np,
                "onesr": onesr,
                "cosT": cosT,
                "sinTs": sinT,
                "masks": masks,
            }
        )
    return in_maps


def kernel(x, cos, sin, W_qkv, W_proj):
    from concourse.bass_utils import run_bass_kernel_spmd

    nc = _get_program()
    in_maps = make_in_maps(x, cos, sin, W_qkv, W_proj)
    trace = bool(int(os.environ.get("KERNEL_TRACE", "0")))
    res = run_bass_kernel_spmd(
        nc, in_maps, core_ids=list(range(NCORES)), trace=trace
    )
    if trace:
        _CACHE["last_results"] = res
        if res.exec_time_ns is not None:
            print(f"HW exec time: {res.exec_time_ns} ns")

    out = np.zeros((B, T, C), dtype=np.float32)
    for core in range(NCORES):
        out[core // 4] += res.results[core]["out"]
    return out



# revision 5
# speedup vs baseline: 1.5189x; 1.5189x over previous
"""Causal self-attention (B=2, T=2048, C=2048, H=16, D=128) on 8 trn2 cores.

Sharding: tensor-parallel over heads x data-parallel over batch.
Core c handles batch c//4, heads [4*(c%4) .. 4*(c%4)+4). Each core computes
qkv projection for its 4 heads, RoPE, causal attention, and a partial
output projection (its heads' rows of W_proj); the host sums the 4 partials
per batch (in fp32; device partials are fp16).

v2 design vs the DRAM-scratch baseline:
  * Everything fp16: matmuls run at full PE rate (like bf16) but with
    ~10x better mantissa than bf16; FWL (fast weight load) applies to
    non-fp32 stationary operands, hiding LDWEIGHTS (~90us exposed in the
    fp32r baseline trace); DVE ops hit the 2x packed mode; DMA halves.
  * Q^T/K^T/V stay SBUF-resident -- no DRAM scratch round trip.
  * Softmax denominator: exp blocks are accumulated into sumP on the DVE
    (fp16, 2x mode); ONE ones-matmul per (head, q-block) contracts the
    partition axis, instead of one matmul per k-block (PE -25us).
  * Diagonal S/PV matmuls shortened: block kb of q-block qb only covers
    q >= kb*128 (N in {128,256,384,512}); mask is multiplicative on P
    after exp (exp can't overflow: scores are O(5)).
  * Phase interleaving: attention wave for t-block tb is emitted with the
    QKV-projection chains of tb+1 (or phase-3 proj tiles, for the last
    wave) woven between its S/PV matmuls, so the ACT exp latency never
    stalls the PE FIFO.

Orientation (all zero-transpose):
  Q^T[d,t] = Wq^T x^T  (RoPE fused on evacuation)   K^T likewise.
  V[t,d]   = x Wv      (natural; lhsT = x^T chunk)
  S^T[k,q] = K^T.T @ Q^T ; P^T = exp(S^T/sqrt(D)) * mask
  O^T[d,q] = V.T @ P^T  (PSUM-accumulated over k-blocks)
  dn[q]    = ones.T @ sumP ; out2T = O^T * recip(dn)
  out[t,c] = sum_hd out2T_hd.T @ Wp_hd
"""

import contextlib
import math
import os
from collections import deque

import numpy as np

B, T, C = 2, 2048, 2048
H, D = 16, 128
HPC = 4  # heads per core
NCORES = 8
KT = C // 128  # 16 contraction tiles
NTB = T // 512  # 4 t-blocks

_CACHE = {}


def _build_program():
    import concourse.tile as tile
    from concourse import bacc, mybir

    f16 = mybir.dt.float16
    f32 = mybir.dt.float32
    Exp = mybir.ActivationFunctionType.Exp
    SCALE = 1.0 / math.sqrt(float(D))

    nc = bacc.Bacc(
        "TRN2", target_bir_lowering=False, debug=False, num_devices=NCORES
    )

    xT = nc.dram_tensor("xT", [C, T], f16, kind="ExternalInput").ap()
    # [p, m, k, c]: m = 8 output M-tiles (4 q heads then 4 k heads)
    wqk = nc.dram_tensor(
        "wqk", [128, 8, KT, 128], f16, kind="ExternalInput"
    ).ap()
    wv = nc.dram_tensor("wv", [128, KT, HPC * D], f16, kind="ExternalInput").ap()
    wp = nc.dram_tensor("wp", [128, HPC, C], f16, kind="ExternalInput").ap()
    cosF = nc.dram_tensor("cosF", [128, T], f16, kind="ExternalInput").ap()
    sinF = nc.dram_tensor("sinF", [128, T], f16, kind="ExternalInput").ap()
    onesd = nc.dram_tensor("onesd", [128, 128], f16, kind="ExternalInput").ap()
    maskd = nc.dram_tensor("maskd", [128, 128], f16, kind="ExternalInput").ap()
    out = nc.dram_tensor("out", [T, C], f16, kind="ExternalOutput").ap()

    with tile.TileContext(nc) as tc:
        with (
            tc.tile_pool(name="consts", bufs=1) as consts,
            tc.tile_pool(name="qkv", bufs=1) as qkvp,
            tc.tile_pool(name="pt", bufs=6) as ptp,
            tc.tile_pool(name="sump", bufs=2) as sumpp,
            tc.tile_pool(name="rb", bufs=2) as rbp,
            tc.tile_pool(name="ob", bufs=4) as obp,
            tc.tile_pool(name="stps", bufs=2, space="PSUM") as stps,
            tc.tile_pool(name="pvps", bufs=2, space="PSUM") as pvps,
            tc.tile_pool(name="dnps", bufs=1, space="PSUM") as dnpsp,
        ):
            es = contextlib.ExitStack()
            p1x = es.enter_context(tc.tile_pool(name="p1x", bufs=2))
            p1w = es.enter_context(tc.tile_pool(name="p1w", bufs=1))
            p1e = es.enter_context(tc.tile_pool(name="p1e", bufs=2))
            p1ps = es.enter_context(
                tc.tile_pool(name="p1ps", bufs=2, space="PSUM")
            )

            # ---------------- persistent SBUF ----------------
            cos_sb = consts.tile([128, T], f16, tag="cos")
            sin_sb = consts.tile([128, T], f16, tag="sin")
            ones_sb = consts.tile([128, 128], f16, tag="ones")
            mask_sb = consts.tile([128, 128], f16, tag="mask")
            wp_sb = consts.tile([128, HPC, C], f16, tag="wp")
            wqkg = p1w.tile([128, 8, KT, 128], f16, tag="wqkg")
            wv_sb = p1w.tile([128, KT, HPC * D], f16, tag="wv")
            qkT = qkvp.tile([128, 8, T], f16, tag="qkT")
            vt = qkvp.tile([128, KT, HPC * D], f16, tag="vt")
            out2T = qkvp.tile([128, HPC, T], f16, tag="out2T")

            # ---------------- helpers ----------------
            def emit_xtb_dma(tb):
                xtb = p1x.tile([128, KT, 512], f16, tag="xtb", name=f"xtb{tb}")
                for k in range(KT):
                    nc.sync.dma_start(
                        out=xtb[:, k],
                        in_=xT[k * 128 : (k + 1) * 128,
                              tb * 512 : (tb + 1) * 512],
                    )
                return xtb

            def rope_evac(ps, m, tb):
                # cross-partition reads must come from PSUM (SB+SB operands
                # are required to share a base partition)
                tsl = slice(tb * 512, (tb + 1) * 512)
                qraw = p1e.tile([128, 512], f16, tag="qraw")
                nc.scalar.copy(qraw, ps)
                t1 = p1e.tile([128, 512], f16, tag="t1")
                nc.vector.tensor_mul(t1[0:64], ps[64:128], sin_sb[0:64, tsl])
                nc.vector.tensor_mul(t1[64:128], ps[0:64],
                                     sin_sb[64:128, tsl])
                gq = p1e.tile([128, 512], f16, tag="gq")
                nc.gpsimd.tensor_mul(gq, qraw, cos_sb[:, tsl])
                nc.vector.tensor_add(qkT[:, m, tsl], gq, t1)

            def p1_steps(tb, xtb):
                """QKV projection for t-block tb; yields every 2 matmuls."""
                for m in range(8):
                    ps = p1ps.tile([128, 512], f32, tag="qk")
                    for k in range(KT):
                        nc.tensor.matmul(
                            ps,
                            lhsT=wqkg[:, m, k, :],
                            rhs=xtb[:, k, :],
                            start=(k == 0),
                            stop=(k == KT - 1),
                        )
                        if k % 2 == 1:
                            if k == KT - 1:
                                rope_evac(ps, m, tb)
                            yield
                for t4 in range(4):
                    psv = p1ps.tile([128, 512], f32, tag="v", bufs=1)
                    for k in range(KT):
                        nc.tensor.matmul(
                            psv,
                            lhsT=xtb[:, k, t4 * 128 : (t4 + 1) * 128],
                            rhs=wv_sb[:, k, :],
                            start=(k == 0),
                            stop=(k == KT - 1),
                        )
                        if k % 2 == 1:
                            if k == KT - 1:
                                nc.scalar.copy(vt[:, tb * 4 + t4, :], psv)
                            yield

            def p3_steps(trange, posp):
                """Output projection tiles; yields every 2 matmuls."""
                for t in trange:
                    tsl = slice(t * 128, (t + 1) * 128)
                    for cb in range(4):
                        csl = slice(cb * 512, (cb + 1) * 512)
                        pos = posp.tile([128, 512], f32, tag="pos")
                        for hd in range(HPC):
                            nc.tensor.matmul(
                                pos,
                                lhsT=out2T[:, hd, tsl],
                                rhs=wp_sb[:, hd, csl],
                                start=(hd == 0),
                                stop=(hd == HPC - 1),
                            )
                            if hd % 2 == 1:
                                if hd == HPC - 1:
                                    ob = obp.tile([128, 512], f16, tag="ob")
                                    nc.vector.tensor_copy(ob, pos)
                                    nc.scalar.dma_start(
                                        out=out[tsl, csl], in_=ob
                                    )
                                yield

            def emit_pv(pv, h, nk, pt, qoff, kb):
                nc.tensor.matmul(
                    pv[:, qoff:],
                    lhsT=vt[:, kb, h * 128 : (h + 1) * 128],
                    rhs=pt[:, qoff:],
                    start=(kb == 0),
                    stop=(kb == nk - 1),
                )

            def emit_wave(tb, filler):
                qsl = slice(tb * 512, (tb + 1) * 512)
                nk = 4 * (tb + 1)
                for h in range(HPC):
                    sumP = sumpp.tile([128, 512], f16, tag="sumP")
                    pv = pvps.tile([128, 512], f32, tag="pv")
                    pending = deque()
                    for kb in range(nk):
                        j = kb - 4 * tb
                        qoff = max(0, j * 128)
                        st = stps.tile([128, 512], f32, tag="st")
                        nc.tensor.matmul(
                            st[:, qoff:],
                            lhsT=qkT[:, 4 + h, kb * 128 : (kb + 1) * 128],
                            rhs=qkT[:, h, tb * 512 + qoff : (tb + 1) * 512],
                            start=True,
                            stop=True,
                        )
                        pt = ptp.tile([128, 512], f16, tag="pt")
                        nc.scalar.activation(
                            pt[:, qoff:], st[:, qoff:], Exp, scale=SCALE
                        )
                        if j >= 0:
                            nc.vector.tensor_mul(
                                pt[:, qoff : qoff + 128],
                                pt[:, qoff : qoff + 128],
                                mask_sb,
                            )
                        if kb == 0:
                            nc.vector.tensor_copy(sumP, pt)
                        else:
                            nc.vector.tensor_add(
                                sumP[:, qoff:], sumP[:, qoff:], pt[:, qoff:]
                            )
                        next(filler, None)
                        pending.append((pt, qoff, kb))
                        if len(pending) >= 3:
                            emit_pv(pv, h, nk, *pending.popleft())
                    while pending:
                        emit_pv(pv, h, nk, *pending.popleft())
                    dn = dnpsp.tile([128, 512], f32, tag="dn")
                    nc.tensor.matmul(
                        dn, lhsT=ones_sb, rhs=sumP, start=True, stop=True
                    )
                    rb = rbp.tile([128, 512], f32, tag="rb")
                    nc.vector.reciprocal_approx_fast(out=rb, in_=dn)
                    nc.vector.tensor_mul(out2T[:, h, qsl], pv, rb)

            # ---------------- DMA preload ----------------
            nc.sync.dma_start(out=wqkg[:, 0], in_=wqk[:, 0])
            xtb0 = p1x.tile([128, KT, 512], f16, tag="xtb", name="xtb0")
            for k in range(KT):
                nc.sync.dma_start(
                    out=xtb0[:, k], in_=xT[k * 128 : (k + 1) * 128, 0:512]
                )
                if k % 2 == 0 and k // 2 + 1 < 8:
                    nc.sync.dma_start(
                        out=wqkg[:, k // 2 + 1], in_=wqk[:, k // 2 + 1]
                    )
            nc.sync.dma_start(out=cos_sb, in_=cosF)
            nc.sync.dma_start(out=sin_sb, in_=sinF)
            for k in range(KT):
                nc.sync.dma_start(out=wv_sb[:, k], in_=wv[:, k])
            nc.sync.dma_start(out=ones_sb, in_=onesd)
            nc.sync.dma_start(out=mask_sb, in_=maskd)
            nc.sync.dma_start(out=wp_sb, in_=wp)

            # ---------------- emission ----------------
            for _ in p1_steps(0, xtb0):
                pass
            xtb_n = emit_xtb_dma(1)
            filler = p1_steps(1, xtb_n)
            for tb in range(NTB):
                if tb == NTB - 1:
                    # all qkv projections done: swap PSUM to phase 3
                    es.close()
                    es3 = contextlib.ExitStack()
                    posp = es3.enter_context(
                        tc.tile_pool(name="posps", bufs=2, space="PSUM")
                    )
                    filler = p3_steps(range(12), posp)
                emit_wave(tb, filler)
                for _ in filler:
                    pass
                if tb + 2 < NTB:
                    xtb_n = emit_xtb_dma(tb + 2)
                    filler = p1_steps(tb + 2, xtb_n)
            for _ in p3_steps(range(12, 16), posp):
                pass
            es3.close()
    nc.compile()
    return nc


def _get_program():
    if "nc" not in _CACHE:
        _CACHE["nc"] = _build_program()
    return _CACHE["nc"]


def make_in_maps(x, cos, sin, W_qkv, W_proj):
    """Host-side sharding: per-core input dicts (numpy, fp16)."""
    f16 = np.float16
    x = np.asarray(x, dtype=np.float32)
    cos = np.asarray(cos, dtype=np.float32)
    sin = np.asarray(sin, dtype=np.float32)
    W_qkv = np.asarray(W_qkv, dtype=np.float32)
    W_proj = np.asarray(W_proj, dtype=np.float32)

    cosF = np.ascontiguousarray(np.tile(cos.T, (2, 1)).astype(f16))
    sinF = np.ascontiguousarray(
        np.concatenate([-sin.T, sin.T], axis=0).astype(f16)
    )
    kl = np.arange(128)[:, None]
    ql = np.arange(128)[None, :]
    mask01 = (ql >= kl).astype(f16)  # [128k, 128q] multiplicative
    ones = np.ones((128, 128), dtype=f16)

    in_maps = []
    for core in range(NCORES):
        b, hg = core // 4, core % 4
        csl = slice(hg * 512, (hg + 1) * 512)
        wqk_cat = np.concatenate(
            [W_qkv[:, csl], W_qkv[:, C + hg * 512 : C + (hg + 1) * 512]],
            axis=1,
        )  # [C, 1024]
        wqk_np = np.ascontiguousarray(
            wqk_cat.reshape(KT, 128, 8, 128).transpose(1, 2, 0, 3).astype(f16)
        )  # [p, m, k, c]
        wv_np = np.ascontiguousarray(
            W_qkv[:, 2 * C + hg * 512 : 2 * C + (hg + 1) * 512]
            .reshape(KT, 128, HPC * D)
            .transpose(1, 0, 2)
            .astype(f16)
        )  # [p, k, c]
        wp_np = np.ascontiguousarray(
            W_proj[hg * 512 : (hg + 1) * 512, :]
            .reshape(HPC, 128, C)
            .transpose(1, 0, 2)
            .astype(f16)
        )  # [p, hd, c]
        xT_np = np.ascontiguousarray(x[b].T.astype(f16))
        in_maps.append(
            {
                "xT": xT_np,
                "wqk": wqk_np,
                "wv": wv_np,
                "wp": wp_np,
                "cosF": cosF,
                "sinF": sinF,
                "onesd": ones,
                "maskd": mask01,
            }
        )
    return in_maps


def kernel(x, cos, sin, W_qkv, W_proj):
    from concourse.bass_utils import run_bass_kernel_spmd

    nc = _get_program()
    in_maps = make_in_maps(x, cos, sin, W_qkv, W_proj)
    trace = bool(int(os.environ.get("KERNEL_TRACE", "0")))
    res = run_bass_kernel_spmd(
        nc, in_maps, core_ids=list(range(NCORES)), trace=trace
    )
    if trace:
        _CACHE["last_results"] = res
        if res.exec_time_ns is not None:
            print(f"HW exec time: {res.exec_time_ns} ns")

    out = np.zeros((B, T, C), dtype=np.float32)
    for core in range(NCORES):
        out[core // 4] += res.results[core]["out"].astype(np.float32)
    return out
